# revision 7
# baseline (speedup 1.0000x reference)
"""DVGRL (graph VAE recsys) forward pass on 8 Trainium2 NeuronCores.

Strategy (self-contained, hardcoded for the problem shapes):
  U=30000 users, I=20000 items, E=64 (2E=128), B=4096, NNZ=SNNZ=960000.

  - Row-shard both sparse graphs by destination user across 8 cores
    (3750 users/core, processed in 30 user-tiles of 128).
  - SpMM per user-tile: dma_gather pulls the 512B weight rows W.T[col]
    for every edge of the tile into SBUF (edges spread across
    partitions), a fused tensor_scalar(is_equal, mult) builds the
    val-scaled one-hot scatter matrix per 128-edge chunk, and the
    TensorEngine accumulates H_tile = sum_j MT_j.T @ X_j in PSUM.
    Bias is folded in as one extra edge per user pointing at a bias row
    appended to the weight table.
  - Batch entries are grouped on host by the core that owns their user
    row, so the decode is fully local per core (no collectives). Padded
    to a uniform PMAX per core; padding rows are dropped on host.
  - Decode: dma_gather the H rows of the core's batch slab, PE-transpose
    to feature-major, compute u_z/s_z (+batch-major copies for output),
    attention MLP on chip, then recon GEMMs as K=65 matmuls (z plus a
    ones-row so the output bias rides in the weight matrix) with
    float32r 500-wide tiles streamed from DRAM, DMA'd straight out.
"""
import sys

if '/opt/trn_rl_repo' not in sys.path:
    sys.path.insert(0, '/opt/trn_rl_repo')

import numpy as np

from concourse import bacc, mybir, tile
from concourse.bass_utils import run_bass_kernel_spmd

NC = 8
U = 30000
I = 20000
E = 64
F = 2 * E          # 128
B = 4096
USH = U // NC      # 3750 users per core
UT = (USH + 127) // 128   # 30 user tiles per core (last has 38 rows)
DT = mybir.dt.float32
DTR = mybir.dt.float32r
I16 = mybir.dt.int16
DEC_N = 500        # free-dim chunk for decode matmuls (<= one PSUM bank)

_cache = {}


def _build(ncA, ncS, PB, trace_friendly=False):
    """Build the SPMD Bass graph.

    ncA/ncS: per-user-tile 128-edge chunk counts (uniform across cores).
    PB: number of 128-row batch chunks per core (PMAXg = 128*PB).
    """
    PMAXg = 128 * PB
    totA = sum(ncA)
    totS = sum(ncS)

    nc = bacc.Bacc("TRN2", target_bir_lowering=False, debug=False, num_devices=NC)

    # --- DRAM parameters (per-core shards / replicated weights) ---
    WqT_d = nc.declare_dram_parameter("WqT", [I + 1, F], DT, isOutput=False)
    WsT_d = nc.declare_dram_parameter("WsT", [U + 1, F], DT, isOutput=False)
    colsA_d = nc.declare_dram_parameter("colsA", [128, totA * 8], I16, isOutput=False)
    colsS_d = nc.declare_dram_parameter("colsS", [128, totS * 8], I16, isOutput=False)
    metaA_d = nc.declare_dram_parameter("metaA", [128, 128 + 2 * totA], DT, isOutput=False)
    metaS_d = nc.declare_dram_parameter("metaS", [128, 128 + 2 * totS], DT, isOutput=False)
    binds_d = nc.declare_dram_parameter("binds", [128, PMAXg // 16], I16, isOutput=False)
    ident_d = nc.declare_dram_parameter("ident", [128, 128], DT, isOutput=False)
    # attention weights packed: [aW1T (128x64) | ab1 col (128x1 pad) | aW2T col]
    attn_d = nc.declare_dram_parameter("attn", [128, E + 2], DT, isOutput=False)
    WpA_d = nc.declare_dram_parameter("WpA", [E + 1, I], DT, isOutput=False)
    WspS_d = nc.declare_dram_parameter("WspS", [E + 1, U], DT, isOutput=False)

    Hq_d = nc.declare_dram_parameter("Hq", [USH, F], DT, isOutput=True)
    Hs_d = nc.declare_dram_parameter("Hs", [USH, F], DT, isOutput=True)
    uz_d = nc.declare_dram_parameter("uz", [PMAXg, E], DT, isOutput=True)
    sz_d = nc.declare_dram_parameter("sz", [PMAXg, E], DT, isOutput=True)
    recA_d = nc.declare_dram_parameter("recA", [PMAXg, I], DT, isOutput=True)
    recS_d = nc.declare_dram_parameter("recS", [PMAXg, U], DT, isOutput=True)

    with tile.TileContext(nc) as tc:
        # ---------------- persistent tables ----------------
        with tc.tile_pool(name="tables", bufs=1) as tpool:
            colsA = tpool.tile([128, totA * 8], I16)
            colsS = tpool.tile([128, totS * 8], I16)
            metaA = tpool.tile([128, 128 + 2 * totA], DT)
            metaS = tpool.tile([128, 128 + 2 * totS], DT)
            ident = tpool.tile([128, 128], DT)
            attn = tpool.tile([128, E + 2], DT)
            binds = tpool.tile([128, PMAXg // 16], I16)
            nc.sync.dma_start(out=colsA[:, :], in_=colsA_d[:, :])
            nc.sync.dma_start(out=colsS[:, :], in_=colsS_d[:, :])
            nc.sync.dma_start(out=metaA[:, :], in_=metaA_d[:, :])
            nc.sync.dma_start(out=metaS[:, :], in_=metaS_d[:, :])
            nc.sync.dma_start(out=ident[:, :], in_=ident_d[:, :])
            nc.sync.dma_start(out=attn[:, :], in_=attn_d[:, :])
            nc.sync.dma_start(out=binds[:, :], in_=binds_d[:, :])

            # ---------------- phase A: the two SpMMs ----------------
            def spmm(ncounts, cols_sb, meta_sb, table_d, hout_d, xpool, mtpool, hpool, psA):
                tot = sum(ncounts)
                iota = meta_sb[:, 0:128]
                e0 = 0
                for t, nch in enumerate(ncounts):
                    X = xpool.tile([128, nch, F], DT, tag="X")
                    nc.gpsimd.dma_gather(
                        X[:, :, :], table_d[:, :],
                        cols_sb[:, e0 * 8:(e0 + nch) * 8],
                        num_idxs=nch * 128, num_idxs_reg=nch * 128,
                        elem_size=F, single_packet=False,
                    )
                    H = psA.tile([128, F], DT, tag="H")
                    for j in range(nch):
                        MT = mtpool.tile([128, 128], DT, tag="MT")
                        nc.vector.tensor_scalar(
                            MT[:, :], iota,
                            meta_sb[:, 128 + e0 + j:128 + e0 + j + 1],
                            meta_sb[:, 128 + tot + e0 + j:128 + tot + e0 + j + 1],
                            mybir.AluOpType.is_equal, mybir.AluOpType.mult,
                        )
                        nc.tensor.matmul(H[:, :], MT[:, :], X[:, j, :],
                                         start=(j == 0), stop=(j == nch - 1))
                    Hsb = hpool.tile([128, F], DT, tag="Hsb")
                    nc.vector.tensor_copy(Hsb[:, :], H[:, :])
                    nu = min(128, USH - t * 128)
                    nc.sync.dma_start(out=hout_d[t * 128:t * 128 + nu, :],
                                      in_=Hsb[:nu, :])
                    e0 += nch

            with (
                tc.tile_pool(name="xg", bufs=2) as xpool,
                tc.tile_pool(name="mt", bufs=4) as mtpool,
                tc.tile_pool(name="hsb", bufs=3) as hpool,
                tc.tile_pool(name="psA", bufs=2, space="PSUM") as psA,
            ):
                spmm(ncA, colsA, metaA, WqT_d, Hq_d, xpool, mtpool, hpool, psA)
                spmm(ncS, colsS, metaS, WsT_d, Hs_d, xpool, mtpool, hpool, psA)

            # ---------------- phase B: batch slab -> z ----------------
            with (
                tc.tile_pool(name="bpool", bufs=1) as bpool,
                tc.tile_pool(name="btmp", bufs=4) as btmp,
                tc.tile_pool(name="psT", bufs=2, space="PSUM") as psT,
                tc.tile_pool(name="psM", bufs=2, space="PSUM") as psM,
            ):
                Gq = bpool.tile([128, PB, F], DT)
                Gs = bpool.tile([128, PB, F], DT)
                nc.gpsimd.dma_gather(Gq[:, :, :], Hq_d[:, :], binds[:, :],
                                     num_idxs=PMAXg, num_idxs_reg=PMAXg,
                                     elem_size=F)
                nc.gpsimd.dma_gather(Gs[:, :, :], Hs_d[:, :], binds[:, :],
                                     num_idxs=PMAXg, num_idxs_reg=PMAXg,
                                     elem_size=F)
                zcat = bpool.tile([128, PMAXg], DT)   # [u_zT ; s_zT]
                uzT = bpool.tile([E, PMAXg], DT)
                szT = bpool.tile([E, PMAXg], DT)
                uzb = bpool.tile([128, PB, E], DT)
                szb = bpool.tile([128, PB, E], DT)
                for bc in range(PB):
                    Tq = psT.tile([128, 128], DT, tag="T")
                    nc.tensor.transpose(Tq[:, :], Gq[:, bc, :], ident[:, :])
                    Ts = psT.tile([128, 128], DT, tag="T")
                    nc.tensor.transpose(Ts[:, :], Gs[:, bc, :], ident[:, :])
                    # u_zT = mu_T + exp(0.5*logvar_T) ; feature-major
                    tq = btmp.tile([E, 128], DT, tag="tq")
                    nc.scalar.activation(tq[:, :], Tq[E:F, :],
                                         mybir.ActivationFunctionType.Exp,
                                         scale=0.5)
                    nc.vector.tensor_add(uzT[:, bc * 128:(bc + 1) * 128],
                                         tq[:, :], Tq[0:E, :])
                    ts = btmp.tile([E, 128], DT, tag="tq")
                    nc.scalar.activation(ts[:, :], Ts[E:F, :],
                                         mybir.ActivationFunctionType.Exp,
                                         scale=0.5)
                    nc.vector.tensor_add(szT[:, bc * 128:(bc + 1) * 128],
                                         ts[:, :], Ts[0:E, :])
                    # batch-major u_z / s_z for the uz/sz outputs
                    tb = btmp.tile([128, E], DT, tag="tb")
                    nc.scalar.activation(tb[:, :], Gq[:, bc, E:F],
                                         mybir.ActivationFunctionType.Exp,
                                         scale=0.5)
                    nc.vector.tensor_add(uzb[:, bc, :], tb[:, :], Gq[:, bc, 0:E])
                    tb2 = btmp.tile([128, E], DT, tag="tb")
                    nc.scalar.activation(tb2[:, :], Gs[:, bc, E:F],
                                         mybir.ActivationFunctionType.Exp,
                                         scale=0.5)
                    nc.vector.tensor_add(szb[:, bc, :], tb2[:, :], Gs[:, bc, 0:E])
                for bc in range(PB):
                    nc.sync.dma_start(out=uz_d[bc * 128:(bc + 1) * 128, :],
                                      in_=uzb[:, bc, :])
                    nc.sync.dma_start(out=sz_d[bc * 128:(bc + 1) * 128, :],
                                      in_=szb[:, bc, :])

                # attention: scoreT = aW2 @ tanh(aW1 @ all_zT + ab1)
                zA = bpool.tile([E + 1, PMAXg], DT)   # [zT ; ones]
                zS = bpool.tile([E + 1, PMAXg], DT)   # [s_zT ; ones]
                nc.vector.memset(zA[E:E + 1, :], 1.0)
                nc.vector.memset(zS[E:E + 1, :], 1.0)
                nc.vector.tensor_copy(zS[0:E, :], szT[:, :])
                nc.vector.tensor_copy(zcat[0:E, :], uzT[:, :])
                nc.vector.tensor_copy(zcat[E:F, :], szT[:, :])
                h1 = bpool.tile([E, PMAXg], DT)
                score = bpool.tile([1, PMAXg], DT)
                for c0 in range(0, PMAXg, 512):
                    cw = min(512, PMAXg - c0)
                    hp = psM.tile([E, 512], DT, tag="ps")
                    nc.tensor.matmul(hp[:, :cw], attn[:, 0:E],
                                     zcat[:, c0:c0 + cw], start=True, stop=True)
                    nc.scalar.activation(h1[:, c0:c0 + cw], hp[:, :cw],
                                         mybir.ActivationFunctionType.Tanh,
                                         bias=attn[0:E, E:E + 1], scale=1.0)
                for c0 in range(0, PMAXg, 512):
                    cw = min(512, PMAXg - c0)
                    sp = psM.tile([1, 512], DT, tag="ps")
                    nc.tensor.matmul(sp[0:1, :cw], attn[0:E, E + 1:E + 2],
                                     h1[:, c0:c0 + cw], start=True, stop=True)
                    nc.vector.tensor_copy(score[0:1, c0:c0 + cw], sp[0:1, :cw])
                # broadcast score over 64 partitions via K=1 matmul with ones
                ones_row = btmp.tile([1, E], DT, tag="ones")
                nc.vector.memset(ones_row[:, :], 1.0)
                d_ = bpool.tile([E, PMAXg], DT)
                nc.vector.tensor_sub(d_[:, :], uzT[:, :], szT[:, :])
                for c0 in range(0, PMAXg, 512):
                    cw = min(512, PMAXg - c0)
                    sb = psM.tile([E, 512], DT, tag="ps")
                    nc.tensor.matmul(sb[:, :cw], ones_row[:, :],
                                     score[0:1, c0:c0 + cw], start=True, stop=True)
                    tmp = btmp.tile([E, 512], DT, tag="tmul")
                    nc.vector.tensor_mul(tmp[:, :cw], sb[:, :cw], d_[:, c0:c0 + cw])
                    nc.vector.tensor_add(zA[0:E, c0:c0 + cw], tmp[:, :cw],
                                         szT[:, c0:c0 + cw])

                # ---------------- phase C: decode GEMMs ----------------
                with (
                    tc.tile_pool(name="wdec", bufs=3) as wdec,
                    tc.tile_pool(name="odec", bufs=4) as odec,
                    tc.tile_pool(name="psD", bufs=4, space="PSUM") as psD,
                ):
                    def decode(w_d, width, z_sb, out_d):
                        for n0 in range(0, width, DEC_N):
                            nw = min(DEC_N, width - n0)
                            wt = wdec.tile([E + 1, DEC_N], DT, tag="w")
                            nc.sync.dma_start(out=wt[:, :nw], in_=w_d[:, n0:n0 + nw])
                            for bc in range(PB):
                                ps = psD.tile([128, DEC_N], DT, tag="d")
                                nc.tensor.matmul(
                                    ps[:, :nw],
                                    z_sb[:, bc * 128:(bc + 1) * 128],
                                    wt[:, :nw], start=True, stop=True)
                                ob = odec.tile([128, DEC_N], DT, tag="o")
                                nc.vector.tensor_copy(ob[:, :nw], ps[:, :nw])
                                nc.sync.dma_start(
                                    out=out_d[bc * 128:(bc + 1) * 128, n0:n0 + nw],
                                    in_=ob[:, :nw])

                    decode(WpA_d, I, zA, recA_d)
                    decode(WspS_d, U, zS, recS_d)

    nc.compile()
    return nc


def _wrap16(idx):
    """int16 index vector -> (128, n/16) wrapped+replicated layout."""
    w = idx.reshape(-1, 16).T.astype(np.int16)
    return np.tile(w, (8, 1))


def _prep_graph(rows, cols, vals, bias_row):
    """Sort edges by (core, user-tile); per tile, pad to a chunk count
    uniform across cores; emit per-core cols/lrow/val tables."""
    core = rows // USH
    lrow_g = rows % USH
    tl = lrow_g // 128
    lr = (lrow_g % 128).astype(np.float32)
    order = np.lexsort((tl, core))
    core, tl, lr = core[order], tl[order], lr[order]
    cols, vals = cols[order], vals[order]

    # counts per (core, tile), including bias edges
    cnt = np.zeros((NC, UT), np.int64)
    np.add.at(cnt, (core, tl), 1)
    nu = np.minimum(128, USH - np.arange(UT) * 128)
    cntb = cnt + nu[None, :]
    nch = ((cntb.max(axis=0) + 127) // 128).astype(np.int64)   # chunks per tile
    tot = int(nch.sum())

    colsT = np.zeros((NC, tot * 128), np.int16)
    lrowT = np.zeros((NC, tot * 128), np.float32)
    valT = np.zeros((NC, tot * 128), np.float32)
    # edge segment starts per (core, tile) in the sorted arrays
    seg = np.zeros((NC, UT + 1), np.int64)
    for c in range(NC):
        m = core == c
        seg[c, 1:] = np.cumsum(np.bincount(tl[m], minlength=UT))
    offs = np.concatenate([[0], np.cumsum(nch * 128)])
    for c in range(NC):
        m = core == c
        ctl, ccols, cvals, clr = tl[m], cols[m], vals[m], lr[m]
        for t in range(UT):
            s, e = seg[c, t], seg[c, t + 1]
            k = e - s
            o = offs[t]
            colsT[c, o:o + k] = ccols[s:e]
            valT[c, o:o + k] = cvals[s:e]
            lrowT[c, o:o + k] = clr[s:e]
            # bias edges
            nb = int(nu[t])
            colsT[c, o + k:o + k + nb] = bias_row
            valT[c, o + k:o + k + nb] = 1.0
            lrowT[c, o + k:o + k + nb] = np.arange(nb, dtype=np.float32)
    return nch, colsT, lrowT, valT


def _pack_meta(nch, lrowT, valT):
    tot = int(nch.sum())
    metas = []
    iota = np.tile(np.arange(128, dtype=np.float32)[None, :], (128, 1))
    for c in range(NC):
        m = np.empty((128, 128 + 2 * tot), np.float32)
        m[:, 0:128] = iota
        m[:, 128:128 + tot] = lrowT[c].reshape(tot, 128).T
        m[:, 128 + tot:] = valT[c].reshape(tot, 128).T
        metas.append(m)
    return metas


def kernel(inputs, graph_row, graph_col, graph_val, s_row, s_col, s_val,
           Wq, bq, Ws, bs, aW1, ab1, aW2, Wp, bp, Wsp, bsp):
    inputs = np.asarray(inputs)
    graph_row = np.asarray(graph_row).astype(np.int64)
    graph_col = np.asarray(graph_col).astype(np.int64)
    graph_val = np.asarray(graph_val, dtype=np.float32)
    s_row = np.asarray(s_row).astype(np.int64)
    s_col = np.asarray(s_col).astype(np.int64)
    s_val = np.asarray(s_val, dtype=np.float32)
    Wq = np.asarray(Wq, dtype=np.float32)
    bq = np.asarray(bq, dtype=np.float32)
    Ws = np.asarray(Ws, dtype=np.float32)
    bs = np.asarray(bs, dtype=np.float32)
    aW1 = np.asarray(aW1, dtype=np.float32)
    ab1 = np.asarray(ab1, dtype=np.float32)
    aW2 = np.asarray(aW2, dtype=np.float32)
    Wp = np.asarray(Wp, dtype=np.float32)
    bp = np.asarray(bp, dtype=np.float32)
    Wsp = np.asarray(Wsp, dtype=np.float32)
    bsp = np.asarray(bsp, dtype=np.float32)

    ncA, colsA, lrowA, valA = _prep_graph(graph_row, graph_col, graph_val, I)
    ncS, colsS, lrowS, valS = _prep_graph(s_row, s_col, s_val, U)
    metasA = _pack_meta(ncA, lrowA, valA)
    metasS = _pack_meta(ncS, lrowS, valS)

    # batch grouping by owner core
    bi = inputs.astype(np.int64)
    owner = bi // USH
    pos_by_core = [np.nonzero(owner == c)[0] for c in range(NC)]
    nmax = max(len(p) for p in pos_by_core)
    PB = (nmax + 127) // 128
    PMAXg = PB * 128

    binds_maps = []
    for c in range(NC):
        lu = np.zeros(PMAXg, np.int16)
        p = pos_by_core[c]
        lu[:len(p)] = (bi[p] - c * USH).astype(np.int16)
        binds_maps.append(_wrap16(lu))

    WqT_aug = np.vstack([Wq.T, bq[None, :]]).astype(np.float32)
    WsT_aug = np.vstack([Ws.T, bs[None, :]]).astype(np.float32)
    WpA = np.vstack([Wp.T, bp[None, :]]).astype(np.float32)
    WspS = np.vstack([Wsp.T, bsp[None, :]]).astype(np.float32)
    ident = np.eye(128, dtype=np.float32)
    attn = np.zeros((128, E + 2), np.float32)
    attn[:, 0:E] = aW1.T            # (2E x E)
    attn[0:E, E] = ab1
    attn[0:E, E + 1] = aW2[0]

    key = (tuple(int(x) for x in ncA), tuple(int(x) for x in ncS), PB)
    if key not in _cache:
        _cache[key] = _build(list(map(int, ncA)), list(map(int, ncS)), PB)
    nc = _cache[key]

    in_maps = []
    for c in range(NC):
        in_maps.append(dict(
            WqT=WqT_aug, WsT=WsT_aug,
            colsA=_wrap16(colsA[c]), colsS=_wrap16(colsS[c]),
            metaA=metasA[c], metaS=metasS[c],
            binds=binds_maps[c], ident=ident, attn=attn,
            WpA=WpA, WspS=WspS,
        ))

    global _last_in_maps
    _last_in_maps = in_maps
    res = run_bass_kernel_spmd(nc, in_maps, core_ids=list(range(NC)))
    results = res.results

    mu = np.concatenate([results[c]["Hq"][:, :E] for c in range(NC)], 0)
    logvar = np.concatenate([results[c]["Hq"][:, E:] for c in range(NC)], 0)
    s_mu = np.concatenate([results[c]["Hs"][:, :E] for c in range(NC)], 0)
    s_logvar = np.concatenate([results[c]["Hs"][:, E:] for c in range(NC)], 0)
    u_z = np.empty((B, E), np.float32)
    s_z = np.empty((B, E), np.float32)
    recon_A = np.empty((B, I), np.float32)
    recon_S = np.empty((B, U), np.float32)
    for c in range(NC):
        p = pos_by_core[c]
        n = len(p)
        u_z[p] = results[c]["uz"][:n]
        s_z[p] = results[c]["sz"][:n]
        recon_A[p] = results[c]["recA"][:n]
        recon_S[p] = results[c]["recS"][:n]
    return (recon_A, recon_S, mu, logvar, s_mu, s_logvar, u_z, s_z)


# revision 9
# speedup vs baseline: 1.6763x; 1.6763x over previous
"""DVGRL (graph VAE recsys) forward pass on 8 Trainium2 NeuronCores.

Strategy (self-contained, hardcoded for the problem shapes):
  U=30000 users, I=20000 items, E=64 (2E=128), B=4096, NNZ=SNNZ=960000.

  - Row-shard both sparse graphs by destination user across 8 cores
    (3750 users/core, processed in 30 user-tiles of 128).
  - SpMM per user-tile: dma_gather pulls the 512B weight rows W.T[col]
    for every edge of the tile into SBUF (edges spread across
    partitions), a fused tensor_scalar(is_equal, mult) builds the
    val-scaled one-hot scatter matrix per 128-edge chunk, and the
    TensorEngine accumulates H_tile = sum_j MT_j.T @ X_j in PSUM.
    Bias is folded in as one extra edge per user pointing at a bias row
    appended to the weight table.
  - Batch entries are grouped on host by the core that owns their user
    row, so the decode is fully local per core (no collectives). Padded
    to a uniform PMAX per core; padding rows are dropped on host.
  - Decode: dma_gather the H rows of the core's batch slab, PE-transpose
    to feature-major, compute u_z/s_z (+batch-major copies for output),
    attention MLP on chip, then recon GEMMs as K=65 matmuls (z plus a
    ones-row so the output bias rides in the weight matrix) with
    float32r 500-wide tiles streamed from DRAM, DMA'd straight out.
"""
import sys

if '/opt/trn_rl_repo' not in sys.path:
    sys.path.insert(0, '/opt/trn_rl_repo')

import numpy as np

from concourse import bacc, mybir, tile
from concourse.bass_utils import run_bass_kernel_spmd

NC = 8
U = 30000
I = 20000
E = 64
F = 2 * E          # 128
B = 4096
USH = U // NC      # 3750 users per core
UT = (USH + 127) // 128   # 30 user tiles per core (last has 38 rows)
DT = mybir.dt.float32
DTR = mybir.dt.float32r
I16 = mybir.dt.int16
DEC_N = 500        # free-dim chunk for decode matmuls (<= one PSUM bank)

_cache = {}


def _build(ncA, ncS, PB, rep=1):
    """Build the SPMD Bass graph.

    ncA/ncS: per-user-tile 128-edge chunk counts (uniform across cores).
    PB: number of 128-row batch chunks per core (PMAXg = 128*PB).
    """
    PMAXg = 128 * PB
    totA = sum(ncA)
    totS = sum(ncS)

    nc = bacc.Bacc("TRN2", target_bir_lowering=False, debug=False, num_devices=NC)

    # --- DRAM parameters (per-core shards / replicated weights) ---
    WqT_d = nc.declare_dram_parameter("WqT", [I + 1, F], DT, isOutput=False)
    WsT_d = nc.declare_dram_parameter("WsT", [U + 1, F], DT, isOutput=False)
    colsA_d = nc.declare_dram_parameter("colsA", [128, totA * 8], I16, isOutput=False)
    colsS_d = nc.declare_dram_parameter("colsS", [128, totS * 8], I16, isOutput=False)
    metaA_d = nc.declare_dram_parameter("metaA", [128, 128 + 2 * totA], DT, isOutput=False)
    metaS_d = nc.declare_dram_parameter("metaS", [128, 128 + 2 * totS], DT, isOutput=False)
    binds_d = nc.declare_dram_parameter("binds", [128, PMAXg // 16], I16, isOutput=False)
    ident_d = nc.declare_dram_parameter("ident", [128, 128], DT, isOutput=False)
    # attention weights packed: [aW1T (128x64) | ab1 col (128x1 pad) | aW2T col]
    attn_d = nc.declare_dram_parameter("attn", [128, E + 2], DT, isOutput=False)
    WpA_d = nc.declare_dram_parameter("WpA", [E + 1, I], DT, isOutput=False)
    WspS_d = nc.declare_dram_parameter("WspS", [E + 1, U], DT, isOutput=False)

    Hq_d = nc.declare_dram_parameter("Hq", [USH, F], DT, isOutput=True)
    Hs_d = nc.declare_dram_parameter("Hs", [USH, F], DT, isOutput=True)
    uz_d = nc.declare_dram_parameter("uz", [PMAXg, E], DT, isOutput=True)
    sz_d = nc.declare_dram_parameter("sz", [PMAXg, E], DT, isOutput=True)
    recA_d = nc.declare_dram_parameter("recA", [PMAXg, I], DT, isOutput=True)
    recS_d = nc.declare_dram_parameter("recS", [PMAXg, U], DT, isOutput=True)

    with tile.TileContext(nc) as tc:
        # ---------------- persistent tables ----------------
        with (
            tc.tile_pool(name="tables", bufs=1) as tpool,
            tc.tile_pool(name="xg", bufs=2) as xpool,
            tc.tile_pool(name="mt", bufs=4) as mtpool,
            tc.tile_pool(name="hsb", bufs=3) as hpool,
            tc.tile_pool(name="psA", bufs=2, space="PSUM") as psA,
            tc.tile_pool(name="bpool", bufs=1) as bpool,
            tc.tile_pool(name="btmp", bufs=4) as btmp,
            tc.tile_pool(name="psT", bufs=2, space="PSUM") as psT,
            tc.tile_pool(name="psM", bufs=2, space="PSUM") as psM,
            tc.tile_pool(name="wdec", bufs=3) as wdec,
            tc.tile_pool(name="odec", bufs=4) as odec,
            tc.tile_pool(name="psD", bufs=2, space="PSUM") as psD,
        ):
            colsA = tpool.tile([128, totA * 8], I16)
            colsS = tpool.tile([128, totS * 8], I16)
            metaA = tpool.tile([128, 128 + 2 * totA], DT)
            metaS = tpool.tile([128, 128 + 2 * totS], DT)
            ident = tpool.tile([128, 128], DT)
            attn = tpool.tile([128, E + 2], DT)
            binds = tpool.tile([128, PMAXg // 16], I16)
            nc.sync.dma_start(out=colsA[:, :], in_=colsA_d[:, :])
            nc.sync.dma_start(out=colsS[:, :], in_=colsS_d[:, :])
            nc.sync.dma_start(out=metaA[:, :], in_=metaA_d[:, :])
            nc.sync.dma_start(out=metaS[:, :], in_=metaS_d[:, :])
            nc.sync.dma_start(out=ident[:, :], in_=ident_d[:, :])
            nc.sync.dma_start(out=attn[:, :], in_=attn_d[:, :])
            nc.sync.dma_start(out=binds[:, :], in_=binds_d[:, :])

            # ---------------- phase A: the two SpMMs ----------------
            def spmm(ncounts, cols_sb, meta_sb, table_d, hout_d):
                tot = sum(ncounts)
                iota = meta_sb[:, 0:128]
                e0 = 0
                for t, nch in enumerate(ncounts):
                    X = xpool.tile([128, nch, F], DT, tag="X")
                    nc.gpsimd.dma_gather(
                        X[:, :, :], table_d[:, :],
                        cols_sb[:, e0 * 8:(e0 + nch) * 8],
                        num_idxs=nch * 128, num_idxs_reg=nch * 128,
                        elem_size=F, single_packet=False,
                    )
                    H = psA.tile([128, F], DT, tag="H")
                    for j in range(nch):
                        MT = mtpool.tile([128, 128], DT, tag="MT")
                        nc.vector.tensor_scalar(
                            MT[:, :], iota,
                            meta_sb[:, 128 + e0 + j:128 + e0 + j + 1],
                            meta_sb[:, 128 + tot + e0 + j:128 + tot + e0 + j + 1],
                            mybir.AluOpType.is_equal, mybir.AluOpType.mult,
                        )
                        nc.tensor.matmul(H[:, :], MT[:, :], X[:, j, :],
                                         start=(j == 0), stop=(j == nch - 1))
                    Hsb = hpool.tile([128, F], DT, tag="Hsb")
                    nc.vector.tensor_copy(Hsb[:, :], H[:, :])
                    nu = min(128, USH - t * 128)
                    nc.sync.dma_start(out=hout_d[t * 128:t * 128 + nu, :],
                                      in_=Hsb[:nu, :])
                    e0 += nch

            def body():
                spmm(ncA, colsA, metaA, WqT_d, Hq_d)
                spmm(ncS, colsS, metaS, WsT_d, Hs_d)

                # ------------ phase B: batch slab -> z ------------
                Gq = bpool.tile([128, PB, F], DT)
                Gs = bpool.tile([128, PB, F], DT)
                nc.gpsimd.dma_gather(Gq[:, :, :], Hq_d[:, :], binds[:, :],
                                     num_idxs=PMAXg, num_idxs_reg=PMAXg,
                                     elem_size=F)
                nc.gpsimd.dma_gather(Gs[:, :, :], Hs_d[:, :], binds[:, :],
                                     num_idxs=PMAXg, num_idxs_reg=PMAXg,
                                     elem_size=F)
                zcat = bpool.tile([128, PMAXg], DT)   # [u_zT ; s_zT]
                uzT = bpool.tile([E, PMAXg], DT)
                szT = bpool.tile([E, PMAXg], DT)
                uzb = bpool.tile([128, PB, E], DT)
                szb = bpool.tile([128, PB, E], DT)
                for bc in range(PB):
                    Tq = psT.tile([128, 128], DT, tag="T")
                    nc.tensor.transpose(Tq[:, :], Gq[:, bc, :], ident[:, :])
                    Ts = psT.tile([128, 128], DT, tag="T")
                    nc.tensor.transpose(Ts[:, :], Gs[:, bc, :], ident[:, :])
                    # u_zT = mu_T + exp(0.5*logvar_T) ; feature-major
                    tq = btmp.tile([E, 128], DT, tag="tq")
                    nc.scalar.activation(tq[:, :], Tq[E:F, :],
                                         mybir.ActivationFunctionType.Exp,
                                         scale=0.5)
                    nc.vector.tensor_add(uzT[:, bc * 128:(bc + 1) * 128],
                                         tq[:, :], Tq[0:E, :])
                    ts = btmp.tile([E, 128], DT, tag="tq")
                    nc.scalar.activation(ts[:, :], Ts[E:F, :],
                                         mybir.ActivationFunctionType.Exp,
                                         scale=0.5)
                    nc.vector.tensor_add(szT[:, bc * 128:(bc + 1) * 128],
                                         ts[:, :], Ts[0:E, :])
                    # batch-major u_z / s_z for the uz/sz outputs
                    tb = btmp.tile([128, E], DT, tag="tb")
                    nc.scalar.activation(tb[:, :], Gq[:, bc, E:F],
                                         mybir.ActivationFunctionType.Exp,
                                         scale=0.5)
                    nc.vector.tensor_add(uzb[:, bc, :], tb[:, :], Gq[:, bc, 0:E])
                    tb2 = btmp.tile([128, E], DT, tag="tb")
                    nc.scalar.activation(tb2[:, :], Gs[:, bc, E:F],
                                         mybir.ActivationFunctionType.Exp,
                                         scale=0.5)
                    nc.vector.tensor_add(szb[:, bc, :], tb2[:, :], Gs[:, bc, 0:E])
                for bc in range(PB):
                    nc.sync.dma_start(out=uz_d[bc * 128:(bc + 1) * 128, :],
                                      in_=uzb[:, bc, :])
                    nc.sync.dma_start(out=sz_d[bc * 128:(bc + 1) * 128, :],
                                      in_=szb[:, bc, :])

                # attention: scoreT = aW2 @ tanh(aW1 @ all_zT + ab1)
                zA = bpool.tile([E + 1, PMAXg], DT)   # [zT ; ones]
                zS = bpool.tile([E + 1, PMAXg], DT)   # [s_zT ; ones]
                nc.vector.memset(zA[E:E + 1, :], 1.0)
                nc.vector.memset(zS[E:E + 1, :], 1.0)
                nc.vector.tensor_copy(zS[0:E, :], szT[:, :])
                nc.vector.tensor_copy(zcat[0:E, :], uzT[:, :])
                nc.vector.tensor_copy(zcat[E:F, :], szT[:, :])
                h1 = bpool.tile([E, PMAXg], DT)
                score = bpool.tile([1, PMAXg], DT)
                for c0 in range(0, PMAXg, 512):
                    cw = min(512, PMAXg - c0)
                    hp = psM.tile([E, 512], DT, tag="ps")
                    nc.tensor.matmul(hp[:, :cw], attn[:, 0:E],
                                     zcat[:, c0:c0 + cw], start=True, stop=True)
                    nc.scalar.activation(h1[:, c0:c0 + cw], hp[:, :cw],
                                         mybir.ActivationFunctionType.Tanh,
                                         bias=attn[0:E, E:E + 1], scale=1.0)
                for c0 in range(0, PMAXg, 512):
                    cw = min(512, PMAXg - c0)
                    sp = psM.tile([1, 512], DT, tag="ps")
                    nc.tensor.matmul(sp[0:1, :cw], attn[0:E, E + 1:E + 2],
                                     h1[:, c0:c0 + cw], start=True, stop=True)
                    nc.vector.tensor_copy(score[0:1, c0:c0 + cw], sp[0:1, :cw])
                # broadcast score over 64 partitions via K=1 matmul with ones
                ones_row = btmp.tile([1, E], DT, tag="ones")
                nc.vector.memset(ones_row[:, :], 1.0)
                d_ = bpool.tile([E, PMAXg], DT)
                nc.vector.tensor_sub(d_[:, :], uzT[:, :], szT[:, :])
                for c0 in range(0, PMAXg, 512):
                    cw = min(512, PMAXg - c0)
                    sb = psM.tile([E, 512], DT, tag="ps")
                    nc.tensor.matmul(sb[:, :cw], ones_row[:, :],
                                     score[0:1, c0:c0 + cw], start=True, stop=True)
                    tmp = btmp.tile([E, 512], DT, tag="tmul")
                    nc.vector.tensor_mul(tmp[:, :cw], sb[:, :cw], d_[:, c0:c0 + cw])
                    nc.vector.tensor_add(zA[0:E, c0:c0 + cw], tmp[:, :cw],
                                         szT[:, c0:c0 + cw])

                # ------------ phase C: decode GEMMs ------------
                def decode(w_d, width, z_sb, out_d):
                    for n0 in range(0, width, DEC_N):
                        nw = min(DEC_N, width - n0)
                        wt = wdec.tile([E + 1, DEC_N], DT, tag="w")
                        nc.sync.dma_start(out=wt[:, :nw], in_=w_d[:, n0:n0 + nw])
                        for bc in range(PB):
                            ps = psD.tile([128, DEC_N], DT, tag="d")
                            nc.tensor.matmul(
                                ps[:, :nw],
                                z_sb[:, bc * 128:(bc + 1) * 128],
                                wt[:, :nw], start=True, stop=True)
                            ob = odec.tile([128, DEC_N], DT, tag="o")
                            nc.vector.tensor_copy(ob[:, :nw], ps[:, :nw])
                            nc.sync.dma_start(
                                out=out_d[bc * 128:(bc + 1) * 128, n0:n0 + nw],
                                in_=ob[:, :nw])

                decode(WpA_d, I, zA, recA_d)
                decode(WspS_d, U, zS, recS_d)

            if rep == 1:
                body()
            else:
                with tc.For_i(0, rep, 1):
                    body()

    nc.compile()
    return nc


def _wrap16(idx):
    """int16 index vector -> (128, n/16) wrapped+replicated layout."""
    w = idx.reshape(-1, 16).T.astype(np.int16)
    return np.tile(w, (8, 1))


def _prep_graph(rows, cols, vals, bias_row):
    """Sort edges by (core, user-tile); per tile, pad to a chunk count
    uniform across cores; emit per-core cols/lrow/val tables."""
    core = rows // USH
    lrow_g = rows % USH
    tl = lrow_g // 128
    lr = (lrow_g % 128).astype(np.float32)
    order = np.lexsort((tl, core))
    core, tl, lr = core[order], tl[order], lr[order]
    cols, vals = cols[order], vals[order]

    # counts per (core, tile), including bias edges
    cnt = np.zeros((NC, UT), np.int64)
    np.add.at(cnt, (core, tl), 1)
    nu = np.minimum(128, USH - np.arange(UT) * 128)
    cntb = cnt + nu[None, :]
    nch = ((cntb.max(axis=0) + 127) // 128).astype(np.int64)   # chunks per tile
    tot = int(nch.sum())

    colsT = np.zeros((NC, tot * 128), np.int16)
    lrowT = np.zeros((NC, tot * 128), np.float32)
    valT = np.zeros((NC, tot * 128), np.float32)
    # edge segment starts per (core, tile) in the sorted arrays
    seg = np.zeros((NC, UT + 1), np.int64)
    for c in range(NC):
        m = core == c
        seg[c, 1:] = np.cumsum(np.bincount(tl[m], minlength=UT))
    offs = np.concatenate([[0], np.cumsum(nch * 128)])
    for c in range(NC):
        m = core == c
        ctl, ccols, cvals, clr = tl[m], cols[m], vals[m], lr[m]
        for t in range(UT):
            s, e = seg[c, t], seg[c, t + 1]
            k = e - s
            o = offs[t]
            colsT[c, o:o + k] = ccols[s:e]
            valT[c, o:o + k] = cvals[s:e]
            lrowT[c, o:o + k] = clr[s:e]
            # bias edges
            nb = int(nu[t])
            colsT[c, o + k:o + k + nb] = bias_row
            valT[c, o + k:o + k + nb] = 1.0
            lrowT[c, o + k:o + k + nb] = np.arange(nb, dtype=np.float32)
    return nch, colsT, lrowT, valT


def _pack_meta(nch, lrowT, valT):
    tot = int(nch.sum())
    metas = []
    iota = np.tile(np.arange(128, dtype=np.float32)[None, :], (128, 1))
    for c in range(NC):
        m = np.empty((128, 128 + 2 * tot), np.float32)
        m[:, 0:128] = iota
        m[:, 128:128 + tot] = lrowT[c].reshape(tot, 128).T
        m[:, 128 + tot:] = valT[c].reshape(tot, 128).T
        metas.append(m)
    return metas


def kernel(inputs, graph_row, graph_col, graph_val, s_row, s_col, s_val,
           Wq, bq, Ws, bs, aW1, ab1, aW2, Wp, bp, Wsp, bsp):
    inputs = np.asarray(inputs)
    graph_row = np.asarray(graph_row).astype(np.int64)
    graph_col = np.asarray(graph_col).astype(np.int64)
    graph_val = np.asarray(graph_val, dtype=np.float32)
    s_row = np.asarray(s_row).astype(np.int64)
    s_col = np.asarray(s_col).astype(np.int64)
    s_val = np.asarray(s_val, dtype=np.float32)
    Wq = np.asarray(Wq, dtype=np.float32)
    bq = np.asarray(bq, dtype=np.float32)
    Ws = np.asarray(Ws, dtype=np.float32)
    bs = np.asarray(bs, dtype=np.float32)
    aW1 = np.asarray(aW1, dtype=np.float32)
    ab1 = np.asarray(ab1, dtype=np.float32)
    aW2 = np.asarray(aW2, dtype=np.float32)
    Wp = np.asarray(Wp, dtype=np.float32)
    bp = np.asarray(bp, dtype=np.float32)
    Wsp = np.asarray(Wsp, dtype=np.float32)
    bsp = np.asarray(bsp, dtype=np.float32)

    ncA, colsA, lrowA, valA = _prep_graph(graph_row, graph_col, graph_val, I)
    ncS, colsS, lrowS, valS = _prep_graph(s_row, s_col, s_val, U)
    metasA = _pack_meta(ncA, lrowA, valA)
    metasS = _pack_meta(ncS, lrowS, valS)

    # batch grouping by owner core
    bi = inputs.astype(np.int64)
    owner = bi // USH
    pos_by_core = [np.nonzero(owner == c)[0] for c in range(NC)]
    nmax = max(len(p) for p in pos_by_core)
    PB = (nmax + 127) // 128
    PMAXg = PB * 128

    binds_maps = []
    for c in range(NC):
        lu = np.zeros(PMAXg, np.int16)
        p = pos_by_core[c]
        lu[:len(p)] = (bi[p] - c * USH).astype(np.int16)
        binds_maps.append(_wrap16(lu))

    WqT_aug = np.vstack([Wq.T, bq[None, :]]).astype(np.float32)
    WsT_aug = np.vstack([Ws.T, bs[None, :]]).astype(np.float32)
    WpA = np.vstack([Wp.T, bp[None, :]]).astype(np.float32)
    WspS = np.vstack([Wsp.T, bsp[None, :]]).astype(np.float32)
    ident = np.eye(128, dtype=np.float32)
    attn = np.zeros((128, E + 2), np.float32)
    attn[:, 0:E] = aW1.T            # (2E x E)
    attn[0:E, E] = ab1
    attn[0:E, E + 1] = aW2[0]

    key = (tuple(int(x) for x in ncA), tuple(int(x) for x in ncS), PB)
    if key not in _cache:
        _cache[key] = _build(list(map(int, ncA)), list(map(int, ncS)), PB)
    nc = _cache[key]

    in_maps = []
    for c in range(NC):
        in_maps.append(dict(
            WqT=WqT_aug, WsT=WsT_aug,
            colsA=_wrap16(colsA[c]), colsS=_wrap16(colsS[c]),
            metaA=metasA[c], metaS=metasS[c],
            binds=binds_maps[c], ident=ident, attn=attn,
            WpA=WpA, WspS=WspS,
        ))

    global _last_in_maps
    _last_in_maps = in_maps
    res = run_bass_kernel_spmd(nc, in_maps, core_ids=list(range(NC)))
    results = res.results

    mu = np.concatenate([results[c]["Hq"][:, :E] for c in range(NC)], 0)
    logvar = np.concatenate([results[c]["Hq"][:, E:] for c in range(NC)], 0)
    s_mu = np.concatenate([results[c]["Hs"][:, :E] for c in range(NC)], 0)
    s_logvar = np.concatenate([results[c]["Hs"][:, E:] for c in range(NC)], 0)
    u_z = np.empty((B, E), np.float32)
    s_z = np.empty((B, E), np.float32)
    recon_A = np.empty((B, I), np.float32)
    recon_S = np.empty((B, U), np.float32)
    for c in range(NC):
        p = pos_by_core[c]
        n = len(p)
        u_z[p] = results[c]["uz"][:n]
        s_z[p] = results[c]["sz"][:n]
        recon_A[p] = results[c]["recA"][:n]
        recon_S[p] = results[c]["recS"][:n]
    return (recon_A, recon_S, mu, logvar, s_mu, s_logvar, u_z, s_z)


# revision 13
# speedup vs baseline: 2.5730x; 1.5350x over previous
"""DVGRL (graph VAE recsys) forward pass on 8 Trainium2 NeuronCores.

Strategy (self-contained, hardcoded for the problem shapes):
  U=30000 users, I=20000 items, E=64 (2E=128), B=4096, NNZ=SNNZ=960000.

  - Row-shard both sparse graphs by destination user across 8 cores
    (3750 users/core, processed in 30 user-tiles of 128).
  - SpMM per user-tile: dma_gather pulls the 512B weight rows W.T[col]
    for every edge of the tile into SBUF (edges spread across
    partitions), a fused tensor_scalar(is_equal, mult) builds the
    val-scaled one-hot scatter matrix per 128-edge chunk, and the
    TensorEngine accumulates H_tile = sum_j MT_j.T @ X_j in PSUM.
    Bias is folded in as one extra edge per user pointing at a bias row
    appended to the weight table.
  - Batch entries are grouped on host by the core that owns their user
    row, so the decode is fully local per core (no collectives). Padded
    to a uniform PMAX per core; padding rows are dropped on host.
  - Decode: dma_gather the H rows of the core's batch slab, PE-transpose
    to feature-major, compute u_z/s_z (+batch-major copies for output),
    attention MLP on chip, then recon GEMMs as K=65 matmuls (z plus a
    ones-row so the output bias rides in the weight matrix) with
    float32r 500-wide tiles streamed from DRAM, DMA'd straight out.
"""
import sys

if '/opt/trn_rl_repo' not in sys.path:
    sys.path.insert(0, '/opt/trn_rl_repo')

import numpy as np

from concourse import bacc, mybir, tile
from concourse.bass_utils import run_bass_kernel_spmd

NC = 8
U = 30000
I = 20000
E = 64
F = 2 * E          # 128
B = 4096
USH = U // NC      # 3750 users per core
UT = (USH + 127) // 128   # 30 user tiles per core (last has 38 rows)
DT = mybir.dt.float32
DTR = mybir.dt.float32r
BF = mybir.dt.bfloat16
I16 = mybir.dt.int16
DEC_N = 500        # free-dim chunk for decode matmuls (<= one PSUM bank)

_cache = {}


def _build(ncA, ncS, PB, rep=1, phases="ABC"):
    """Build the SPMD Bass graph.

    ncA/ncS: per-user-tile 128-edge chunk counts (uniform across cores).
    PB: number of 128-row batch chunks per core (PMAXg = 128*PB).
    """
    PMAXg = 128 * PB
    totA = sum(ncA)
    totS = sum(ncS)

    nc = bacc.Bacc("TRN2", target_bir_lowering=False, debug=False, num_devices=NC)

    # --- DRAM parameters (per-core shards / replicated weights) ---
    WqT_d = nc.declare_dram_parameter("WqT", [I + 1, F], BF, isOutput=False)
    WsT_d = nc.declare_dram_parameter("WsT", [U + 1, F], BF, isOutput=False)
    colsA_d = nc.declare_dram_parameter("colsA", [128, totA * 8], I16, isOutput=False)
    colsS_d = nc.declare_dram_parameter("colsS", [128, totS * 8], I16, isOutput=False)
    metaA_d = nc.declare_dram_parameter("metaA", [128, 128 + 2 * totA], DT, isOutput=False)
    metaS_d = nc.declare_dram_parameter("metaS", [128, 128 + 2 * totS], DT, isOutput=False)
    binds_d = nc.declare_dram_parameter("binds", [128, PMAXg // 16], I16, isOutput=False)
    ident_d = nc.declare_dram_parameter("ident", [128, 128], DT, isOutput=False)
    # attention weights packed: [aW1T (128x64) | ab1 col (128x1 pad) | aW2T col]
    attn_d = nc.declare_dram_parameter("attn", [128, E + 2], DT, isOutput=False)
    WpA_d = nc.declare_dram_parameter("WpA", [E + 1, I], DT, isOutput=False)
    WspS_d = nc.declare_dram_parameter("WspS", [E + 1, U], DT, isOutput=False)

    Hq_d = nc.declare_dram_parameter("Hq", [USH, F], DT, isOutput=True)
    Hs_d = nc.declare_dram_parameter("Hs", [USH, F], DT, isOutput=True)
    uz_d = nc.declare_dram_parameter("uz", [PMAXg, E], DT, isOutput=True)
    sz_d = nc.declare_dram_parameter("sz", [PMAXg, E], DT, isOutput=True)
    recA_d = nc.declare_dram_parameter("recA", [PMAXg, I], DT, isOutput=True)
    recS_d = nc.declare_dram_parameter("recS", [PMAXg, U], DT, isOutput=True)

    with tile.TileContext(nc) as tc:
        # ---------------- persistent tables ----------------
        with (
            tc.tile_pool(name="tables", bufs=1) as tpool,
            tc.tile_pool(name="xg", bufs=2) as xpool,
            tc.tile_pool(name="mt", bufs=4) as mtpool,
            tc.tile_pool(name="hsb", bufs=3) as hpool,
            tc.tile_pool(name="psA", bufs=2, space="PSUM") as psA,
            tc.tile_pool(name="bpool", bufs=1) as bpool,
            tc.tile_pool(name="btmp", bufs=4) as btmp,
            tc.tile_pool(name="psT", bufs=2, space="PSUM") as psT,
            tc.tile_pool(name="psM", bufs=2, space="PSUM") as psM,
            tc.tile_pool(name="wdec", bufs=3) as wdec,
            tc.tile_pool(name="odec", bufs=4) as odec,
            tc.tile_pool(name="psD", bufs=2, space="PSUM") as psD,
        ):
            colsA = tpool.tile([128, totA * 8], I16)
            colsS = tpool.tile([128, totS * 8], I16)
            metaA = tpool.tile([128, 128 + 2 * totA], DT)
            metaS = tpool.tile([128, 128 + 2 * totS], DT)
            ident = tpool.tile([128, 128], DT)
            attn = tpool.tile([128, E + 2], DT)
            binds = tpool.tile([128, PMAXg // 16], I16)
            nc.sync.dma_start(out=colsA[:, :], in_=colsA_d[:, :])
            nc.sync.dma_start(out=colsS[:, :], in_=colsS_d[:, :])
            nc.sync.dma_start(out=metaA[:, :], in_=metaA_d[:, :])
            nc.sync.dma_start(out=metaS[:, :], in_=metaS_d[:, :])
            nc.sync.dma_start(out=ident[:, :], in_=ident_d[:, :])
            nc.sync.dma_start(out=attn[:, :], in_=attn_d[:, :])
            nc.sync.dma_start(out=binds[:, :], in_=binds_d[:, :])

            # ---------------- phase A: the two SpMMs ----------------
            def spmm(ncounts, cols_sb, meta_sb, table_d, hout_d):
                tot = sum(ncounts)
                iota = meta_sb[:, 0:128]
                e0 = 0
                for t, nch in enumerate(ncounts):
                    X = xpool.tile([128, nch, F], BF, tag="X")
                    nc.gpsimd.dma_gather(
                        X[:, :, :], table_d[:, :],
                        cols_sb[:, e0 * 8:(e0 + nch) * 8],
                        num_idxs=nch * 128, num_idxs_reg=nch * 128,
                        elem_size=F, single_packet=False,
                    )
                    H = psA.tile([128, F], DT, tag="H")
                    for j in range(nch):
                        MT = mtpool.tile([128, 128], BF, tag="MT")
                        nc.vector.tensor_scalar(
                            MT[:, :], iota,
                            meta_sb[:, 128 + e0 + j:128 + e0 + j + 1],
                            meta_sb[:, 128 + tot + e0 + j:128 + tot + e0 + j + 1],
                            mybir.AluOpType.is_equal, mybir.AluOpType.mult,
                        )
                        nc.tensor.matmul(H[:, :], MT[:, :], X[:, j, :],
                                         start=(j == 0), stop=(j == nch - 1))
                    Hsb = hpool.tile([128, F], DT, tag="Hsb")
                    nc.vector.tensor_copy(Hsb[:, :], H[:, :])
                    nu = min(128, USH - t * 128)
                    nc.sync.dma_start(out=hout_d[t * 128:t * 128 + nu, :],
                                      in_=Hsb[:nu, :])
                    e0 += nch

            def body():
                if "A" in phases:
                    spmm(ncA, colsA, metaA, WqT_d, Hq_d)
                    spmm(ncS, colsS, metaS, WsT_d, Hs_d)
                if "B" not in phases and "b" not in phases:
                    if "C" in phases:
                        zA = bpool.tile([E + 1, PMAXg], DT)
                        zS = bpool.tile([E + 1, PMAXg], DT)
                        nc.vector.memset(zA[:, :], 0.5)
                        nc.vector.memset(zS[:, :], 0.5)
                        decode_all(zA, zS)
                    return

                # ------------ phase B: batch slab -> z ------------
                Gq = bpool.tile([128, PB, F], DT)
                Gs = bpool.tile([128, PB, F], DT)
                if "b" in phases:
                    nc.vector.memset(Gq[:, :, :], 0.25)
                    nc.vector.memset(Gs[:, :, :], 0.25)
                else:
                    nc.gpsimd.dma_gather(Gq[:, :, :], Hq_d[:, :], binds[:, :],
                                         num_idxs=PMAXg, num_idxs_reg=PMAXg,
                                         elem_size=F)
                    nc.gpsimd.dma_gather(Gs[:, :, :], Hs_d[:, :], binds[:, :],
                                         num_idxs=PMAXg, num_idxs_reg=PMAXg,
                                         elem_size=F)
                zcat = bpool.tile([128, PMAXg], DT)   # [u_zT ; s_zT]
                uzT = bpool.tile([E, PMAXg], DT)
                szT = bpool.tile([E, PMAXg], DT)
                uzb = bpool.tile([128, PB, E], DT)
                szb = bpool.tile([128, PB, E], DT)
                for bc in range(PB):
                    Tq = psT.tile([128, 128], DT, tag="T")
                    nc.tensor.transpose(Tq[:, :], Gq[:, bc, :], ident[:, :])
                    Ts = psT.tile([128, 128], DT, tag="T")
                    nc.tensor.transpose(Ts[:, :], Gs[:, bc, :], ident[:, :])
                    # u_zT = mu_T + exp(0.5*logvar_T) ; feature-major
                    tq = btmp.tile([E, 128], DT, tag="tq")
                    nc.scalar.activation(tq[:, :], Tq[E:F, :],
                                         mybir.ActivationFunctionType.Exp,
                                         scale=0.5)
                    nc.vector.tensor_add(uzT[:, bc * 128:(bc + 1) * 128],
                                         tq[:, :], Tq[0:E, :])
                    ts = btmp.tile([E, 128], DT, tag="tq")
                    nc.scalar.activation(ts[:, :], Ts[E:F, :],
                                         mybir.ActivationFunctionType.Exp,
                                         scale=0.5)
                    nc.vector.tensor_add(szT[:, bc * 128:(bc + 1) * 128],
                                         ts[:, :], Ts[0:E, :])
                    # batch-major u_z / s_z for the uz/sz outputs
                    tb = btmp.tile([128, E], DT, tag="tb")
                    nc.scalar.activation(tb[:, :], Gq[:, bc, E:F],
                                         mybir.ActivationFunctionType.Exp,
                                         scale=0.5)
                    nc.vector.tensor_add(uzb[:, bc, :], tb[:, :], Gq[:, bc, 0:E])
                    tb2 = btmp.tile([128, E], DT, tag="tb")
                    nc.scalar.activation(tb2[:, :], Gs[:, bc, E:F],
                                         mybir.ActivationFunctionType.Exp,
                                         scale=0.5)
                    nc.vector.tensor_add(szb[:, bc, :], tb2[:, :], Gs[:, bc, 0:E])
                for bc in range(PB):
                    nc.sync.dma_start(out=uz_d[bc * 128:(bc + 1) * 128, :],
                                      in_=uzb[:, bc, :])
                    nc.sync.dma_start(out=sz_d[bc * 128:(bc + 1) * 128, :],
                                      in_=szb[:, bc, :])

                # attention: scoreT = aW2 @ tanh(aW1 @ all_zT + ab1)
                zA = bpool.tile([E + 1, PMAXg], DT)   # [zT ; ones]
                zS = bpool.tile([E + 1, PMAXg], DT)   # [s_zT ; ones]
                nc.vector.memset(zA[E:E + 1, :], 1.0)
                nc.vector.memset(zS[E:E + 1, :], 1.0)
                nc.vector.tensor_copy(zS[0:E, :], szT[:, :])
                nc.vector.tensor_copy(zcat[0:E, :], uzT[:, :])
                nc.vector.tensor_copy(zcat[E:F, :], szT[:, :])
                h1 = bpool.tile([E, PMAXg], DT)
                score = bpool.tile([1, PMAXg], DT)
                for c0 in range(0, PMAXg, 512):
                    cw = min(512, PMAXg - c0)
                    hp = psM.tile([E, 512], DT, tag="ps")
                    nc.tensor.matmul(hp[:, :cw], attn[:, 0:E],
                                     zcat[:, c0:c0 + cw], start=True, stop=True)
                    nc.scalar.activation(h1[:, c0:c0 + cw], hp[:, :cw],
                                         mybir.ActivationFunctionType.Tanh,
                                         bias=attn[0:E, E:E + 1], scale=1.0)
                for c0 in range(0, PMAXg, 512):
                    cw = min(512, PMAXg - c0)
                    sp = psM.tile([1, 512], DT, tag="ps")
                    nc.tensor.matmul(sp[0:1, :cw], attn[0:E, E + 1:E + 2],
                                     h1[:, c0:c0 + cw], start=True, stop=True)
                    nc.vector.tensor_copy(score[0:1, c0:c0 + cw], sp[0:1, :cw])
                # broadcast score over 64 partitions via K=1 matmul with ones
                ones_row = btmp.tile([1, E], DT, tag="ones")
                nc.vector.memset(ones_row[:, :], 1.0)
                d_ = bpool.tile([E, PMAXg], DT)
                nc.vector.tensor_sub(d_[:, :], uzT[:, :], szT[:, :])
                for c0 in range(0, PMAXg, 512):
                    cw = min(512, PMAXg - c0)
                    sb = psM.tile([E, 512], DT, tag="ps")
                    nc.tensor.matmul(sb[:, :cw], ones_row[:, :],
                                     score[0:1, c0:c0 + cw], start=True, stop=True)
                    tmp = btmp.tile([E, 512], DT, tag="tmul")
                    nc.vector.tensor_mul(tmp[:, :cw], sb[:, :cw], d_[:, c0:c0 + cw])
                    nc.vector.tensor_add(zA[0:E, c0:c0 + cw], tmp[:, :cw],
                                         szT[:, c0:c0 + cw])

                # ------------ phase C: decode GEMMs ------------
                if "C" in phases:
                    decode_all(zA, zS)

            def decode_all(zA, zS):
                def decode(w_d, width, z_sb, out_d):
                    for n0 in range(0, width, DEC_N):
                        nw = min(DEC_N, width - n0)
                        wt = wdec.tile([E + 1, DEC_N], DT, tag="w")
                        nc.sync.dma_start(out=wt[:, :nw], in_=w_d[:, n0:n0 + nw])
                        for bc in range(PB):
                            ps = psD.tile([128, DEC_N], DT, tag="d")
                            nc.tensor.matmul(
                                ps[:, :nw],
                                z_sb[:, bc * 128:(bc + 1) * 128],
                                wt[:, :nw], start=True, stop=True)
                            ob = odec.tile([128, DEC_N], DT, tag="o")
                            nc.vector.tensor_copy(ob[:, :nw], ps[:, :nw])
                            nc.sync.dma_start(
                                out=out_d[bc * 128:(bc + 1) * 128, n0:n0 + nw],
                                in_=ob[:, :nw])

                decode(WpA_d, I, zA, recA_d)
                decode(WspS_d, U, zS, recS_d)

            if rep == 1:
                body()
            else:
                with tc.For_i(0, rep, 1):
                    body()

    nc.compile()
    return nc


def _wrap16(idx):
    """int16 index vector -> (128, n/16) wrapped+replicated layout."""
    w = idx.reshape(-1, 16).T.astype(np.int16)
    return np.tile(w, (8, 1))


def _prep_graph(rows, cols, vals, bias_row):
    """Sort edges by (core, user-tile); per tile, pad to a chunk count
    uniform across cores; emit per-core cols/lrow/val tables."""
    core = rows // USH
    lrow_g = rows % USH
    tl = lrow_g // 128
    lr = (lrow_g % 128).astype(np.float32)
    order = np.lexsort((tl, core))
    core, tl, lr = core[order], tl[order], lr[order]
    cols, vals = cols[order], vals[order]

    # counts per (core, tile), including bias edges
    cnt = np.zeros((NC, UT), np.int64)
    np.add.at(cnt, (core, tl), 1)
    nu = np.minimum(128, USH - np.arange(UT) * 128)
    cntb = cnt + nu[None, :]
    nch = ((cntb.max(axis=0) + 127) // 128).astype(np.int64)   # chunks per tile
    tot = int(nch.sum())

    colsT = np.zeros((NC, tot * 128), np.int16)
    lrowT = np.zeros((NC, tot * 128), np.float32)
    valT = np.zeros((NC, tot * 128), np.float32)
    # edge segment starts per (core, tile) in the sorted arrays
    seg = np.zeros((NC, UT + 1), np.int64)
    for c in range(NC):
        m = core == c
        seg[c, 1:] = np.cumsum(np.bincount(tl[m], minlength=UT))
    offs = np.concatenate([[0], np.cumsum(nch * 128)])
    for c in range(NC):
        m = core == c
        ctl, ccols, cvals, clr = tl[m], cols[m], vals[m], lr[m]
        for t in range(UT):
            s, e = seg[c, t], seg[c, t + 1]
            k = e - s
            o = offs[t]
            colsT[c, o:o + k] = ccols[s:e]
            valT[c, o:o + k] = cvals[s:e]
            lrowT[c, o:o + k] = clr[s:e]
            # bias edges
            nb = int(nu[t])
            colsT[c, o + k:o + k + nb] = bias_row
            valT[c, o + k:o + k + nb] = 1.0
            lrowT[c, o + k:o + k + nb] = np.arange(nb, dtype=np.float32)
    return nch, colsT, lrowT, valT


def _pack_meta(nch, lrowT, valT):
    tot = int(nch.sum())
    metas = []
    iota = np.tile(np.arange(128, dtype=np.float32)[None, :], (128, 1))
    for c in range(NC):
        m = np.empty((128, 128 + 2 * tot), np.float32)
        m[:, 0:128] = iota
        m[:, 128:128 + tot] = lrowT[c].reshape(tot, 128).T
        m[:, 128 + tot:] = valT[c].reshape(tot, 128).T
        metas.append(m)
    return metas


def kernel(inputs, graph_row, graph_col, graph_val, s_row, s_col, s_val,
           Wq, bq, Ws, bs, aW1, ab1, aW2, Wp, bp, Wsp, bsp):
    inputs = np.asarray(inputs)
    graph_row = np.asarray(graph_row).astype(np.int64)
    graph_col = np.asarray(graph_col).astype(np.int64)
    graph_val = np.asarray(graph_val, dtype=np.float32)
    s_row = np.asarray(s_row).astype(np.int64)
    s_col = np.asarray(s_col).astype(np.int64)
    s_val = np.asarray(s_val, dtype=np.float32)
    Wq = np.asarray(Wq, dtype=np.float32)
    bq = np.asarray(bq, dtype=np.float32)
    Ws = np.asarray(Ws, dtype=np.float32)
    bs = np.asarray(bs, dtype=np.float32)
    aW1 = np.asarray(aW1, dtype=np.float32)
    ab1 = np.asarray(ab1, dtype=np.float32)
    aW2 = np.asarray(aW2, dtype=np.float32)
    Wp = np.asarray(Wp, dtype=np.float32)
    bp = np.asarray(bp, dtype=np.float32)
    Wsp = np.asarray(Wsp, dtype=np.float32)
    bsp = np.asarray(bsp, dtype=np.float32)

    ncA, colsA, lrowA, valA = _prep_graph(graph_row, graph_col, graph_val, I)
    ncS, colsS, lrowS, valS = _prep_graph(s_row, s_col, s_val, U)
    metasA = _pack_meta(ncA, lrowA, valA)
    metasS = _pack_meta(ncS, lrowS, valS)

    # batch grouping by owner core
    bi = inputs.astype(np.int64)
    owner = bi // USH
    pos_by_core = [np.nonzero(owner == c)[0] for c in range(NC)]
    nmax = max(len(p) for p in pos_by_core)
    PB = (nmax + 127) // 128
    PMAXg = PB * 128

    binds_maps = []
    for c in range(NC):
        lu = np.zeros(PMAXg, np.int16)
        p = pos_by_core[c]
        lu[:len(p)] = (bi[p] - c * USH).astype(np.int16)
        binds_maps.append(_wrap16(lu))

    import ml_dtypes
    WqT_aug = np.vstack([Wq.T, bq[None, :]]).astype(ml_dtypes.bfloat16)
    WsT_aug = np.vstack([Ws.T, bs[None, :]]).astype(ml_dtypes.bfloat16)
    WpA = np.vstack([Wp.T, bp[None, :]]).astype(np.float32)
    WspS = np.vstack([Wsp.T, bsp[None, :]]).astype(np.float32)
    ident = np.eye(128, dtype=np.float32)
    attn = np.zeros((128, E + 2), np.float32)
    attn[:, 0:E] = aW1.T            # (2E x E)
    attn[0:E, E] = ab1
    attn[0:E, E + 1] = aW2[0]

    key = (tuple(int(x) for x in ncA), tuple(int(x) for x in ncS), PB)
    if key not in _cache:
        _cache[key] = _build(list(map(int, ncA)), list(map(int, ncS)), PB)
    nc = _cache[key]

    in_maps = []
    for c in range(NC):
        in_maps.append(dict(
            WqT=WqT_aug, WsT=WsT_aug,
            colsA=_wrap16(colsA[c]), colsS=_wrap16(colsS[c]),
            metaA=metasA[c], metaS=metasS[c],
            binds=binds_maps[c], ident=ident, attn=attn,
            WpA=WpA, WspS=WspS,
        ))

    global _last_in_maps
    _last_in_maps = in_maps
    res = run_bass_kernel_spmd(nc, in_maps, core_ids=list(range(NC)))
    results = res.results

    mu = np.concatenate([results[c]["Hq"][:, :E] for c in range(NC)], 0)
    logvar = np.concatenate([results[c]["Hq"][:, E:] for c in range(NC)], 0)
    s_mu = np.concatenate([results[c]["Hs"][:, :E] for c in range(NC)], 0)
    s_logvar = np.concatenate([results[c]["Hs"][:, E:] for c in range(NC)], 0)
    u_z = np.empty((B, E), np.float32)
    s_z = np.empty((B, E), np.float32)
    recon_A = np.empty((B, I), np.float32)
    recon_S = np.empty((B, U), np.float32)
    for c in range(NC):
        p = pos_by_core[c]
        n = len(p)
        u_z[p] = results[c]["uz"][:n]
        s_z[p] = results[c]["sz"][:n]
        recon_A[p] = results[c]["recA"][:n]
        recon_S[p] = results[c]["recS"][:n]
    return (recon_A, recon_S, mu, logvar, s_mu, s_logvar, u_z, s_z)


# revision 14
# speedup vs baseline: 3.0329x; 1.1787x over previous
"""DVGRL (graph VAE recsys) forward pass on 8 Trainium2 NeuronCores.

Strategy (self-contained, hardcoded for the problem shapes):
  U=30000 users, I=20000 items, E=64 (2E=128), B=4096, NNZ=SNNZ=960000.

  - Row-shard both sparse graphs by destination user across 8 cores
    (3750 users/core, processed in 30 user-tiles of 128).
  - SpMM per user-tile: dma_gather pulls the 512B weight rows W.T[col]
    for every edge of the tile into SBUF (edges spread across
    partitions), a fused tensor_scalar(is_equal, mult) builds the
    val-scaled one-hot scatter matrix per 128-edge chunk, and the
    TensorEngine accumulates H_tile = sum_j MT_j.T @ X_j in PSUM.
    Bias is folded in as one extra edge per user pointing at a bias row
    appended to the weight table.
  - Batch entries are grouped on host by the core that owns their user
    row, so the decode is fully local per core (no collectives). Padded
    to a uniform PMAX per core; padding rows are dropped on host.
  - Decode: dma_gather the H rows of the core's batch slab, PE-transpose
    to feature-major, compute u_z/s_z (+batch-major copies for output),
    attention MLP on chip, then recon GEMMs as K=65 matmuls (z plus a
    ones-row so the output bias rides in the weight matrix) with
    float32r 500-wide tiles streamed from DRAM, DMA'd straight out.
"""
import sys

if '/opt/trn_rl_repo' not in sys.path:
    sys.path.insert(0, '/opt/trn_rl_repo')

import numpy as np

from concourse import bacc, mybir, tile
from concourse.bass_utils import run_bass_kernel_spmd

NC = 8
U = 30000
I = 20000
E = 64
F = 2 * E          # 128
B = 4096
USH = U // NC      # 3750 users per core
UT = (USH + 127) // 128   # 30 user tiles per core (last has 38 rows)
DT = mybir.dt.float32
DTR = mybir.dt.float32r
BF = mybir.dt.bfloat16
I16 = mybir.dt.int16
DEC_N = 500        # free-dim chunk for decode matmuls (<= one PSUM bank)

_cache = {}


def _build(ncA, ncS, PB, rep=1, phases="ABC"):
    """Build the SPMD Bass graph.

    ncA/ncS: per-user-tile 128-edge chunk counts (uniform across cores).
    PB: number of 128-row batch chunks per core (PMAXg = 128*PB).
    """
    PMAXg = 128 * PB
    totA = sum(ncA)
    totS = sum(ncS)

    nc = bacc.Bacc("TRN2", target_bir_lowering=False, debug=False, num_devices=NC)

    # --- DRAM parameters (per-core shards / replicated weights) ---
    WqT_d = nc.declare_dram_parameter("WqT", [I + 1, F], BF, isOutput=False)
    WsT_d = nc.declare_dram_parameter("WsT", [U + 1, F], BF, isOutput=False)
    colsA_d = nc.declare_dram_parameter("colsA", [128, totA * 8], I16, isOutput=False)
    colsS_d = nc.declare_dram_parameter("colsS", [128, totS * 8], I16, isOutput=False)
    metaA_d = nc.declare_dram_parameter("metaA", [128, 128 + 2 * totA], DT, isOutput=False)
    metaS_d = nc.declare_dram_parameter("metaS", [128, 128 + 2 * totS], DT, isOutput=False)
    binds_d = nc.declare_dram_parameter("binds", [128, PMAXg // 16], I16, isOutput=False)
    ident_d = nc.declare_dram_parameter("ident", [128, 128], DT, isOutput=False)
    # attention weights packed: [aW1T (128x64) | ab1 col (128x1 pad) | aW2T col]
    attn_d = nc.declare_dram_parameter("attn", [128, E + 2], DT, isOutput=False)
    WpA_d = nc.declare_dram_parameter("WpA", [E + 1, I], BF, isOutput=False)
    WspS_d = nc.declare_dram_parameter("WspS", [E + 1, U], BF, isOutput=False)

    Hq_d = nc.declare_dram_parameter("Hq", [USH, F], DT, isOutput=True)
    Hs_d = nc.declare_dram_parameter("Hs", [USH, F], DT, isOutput=True)
    uz_d = nc.declare_dram_parameter("uz", [PMAXg, E], DT, isOutput=True)
    sz_d = nc.declare_dram_parameter("sz", [PMAXg, E], DT, isOutput=True)
    recA_d = nc.declare_dram_parameter("recA", [PMAXg, I], DT, isOutput=True)
    recS_d = nc.declare_dram_parameter("recS", [PMAXg, U], DT, isOutput=True)

    with tile.TileContext(nc) as tc:
        # ---------------- persistent tables ----------------
        with (
            tc.tile_pool(name="tables", bufs=1) as tpool,
            tc.tile_pool(name="xg", bufs=2) as xpool,
            tc.tile_pool(name="mt", bufs=4) as mtpool,
            tc.tile_pool(name="hsb", bufs=3) as hpool,
            tc.tile_pool(name="psA", bufs=2, space="PSUM") as psA,
            tc.tile_pool(name="bpool", bufs=1) as bpool,
            tc.tile_pool(name="btmp", bufs=4) as btmp,
            tc.tile_pool(name="psT", bufs=2, space="PSUM") as psT,
            tc.tile_pool(name="psM", bufs=2, space="PSUM") as psM,
            tc.tile_pool(name="wdec", bufs=3) as wdec,
            tc.tile_pool(name="odec", bufs=4) as odec,
            tc.tile_pool(name="psD", bufs=2, space="PSUM") as psD,
        ):
            colsA = tpool.tile([128, totA * 8], I16)
            colsS = tpool.tile([128, totS * 8], I16)
            metaA = tpool.tile([128, 128 + 2 * totA], DT)
            metaS = tpool.tile([128, 128 + 2 * totS], DT)
            ident = tpool.tile([128, 128], DT)
            attn = tpool.tile([128, E + 2], DT)
            binds = tpool.tile([128, PMAXg // 16], I16)
            nc.sync.dma_start(out=colsA[:, :], in_=colsA_d[:, :])
            nc.sync.dma_start(out=colsS[:, :], in_=colsS_d[:, :])
            nc.sync.dma_start(out=metaA[:, :], in_=metaA_d[:, :])
            nc.sync.dma_start(out=metaS[:, :], in_=metaS_d[:, :])
            nc.sync.dma_start(out=ident[:, :], in_=ident_d[:, :])
            nc.sync.dma_start(out=attn[:, :], in_=attn_d[:, :])
            nc.sync.dma_start(out=binds[:, :], in_=binds_d[:, :])

            # ---------------- phase A: the two SpMMs ----------------
            def spmm(ncounts, cols_sb, meta_sb, table_d, hout_d):
                tot = sum(ncounts)
                iota = meta_sb[:, 0:128]
                e0 = 0
                for t, nch in enumerate(ncounts):
                    X = xpool.tile([128, nch, F], BF, tag="X")
                    nc.gpsimd.dma_gather(
                        X[:, :, :], table_d[:, :],
                        cols_sb[:, e0 * 8:(e0 + nch) * 8],
                        num_idxs=nch * 128, num_idxs_reg=nch * 128,
                        elem_size=F, single_packet=False,
                    )
                    H = psA.tile([128, F], DT, tag="H")
                    for j in range(nch):
                        MT = mtpool.tile([128, 128], BF, tag="MT")
                        nc.vector.tensor_scalar(
                            MT[:, :], iota,
                            meta_sb[:, 128 + e0 + j:128 + e0 + j + 1],
                            meta_sb[:, 128 + tot + e0 + j:128 + tot + e0 + j + 1],
                            mybir.AluOpType.is_equal, mybir.AluOpType.mult,
                        )
                        nc.tensor.matmul(H[:, :], MT[:, :], X[:, j, :],
                                         start=(j == 0), stop=(j == nch - 1))
                    Hsb = hpool.tile([128, F], DT, tag="Hsb")
                    nc.vector.tensor_copy(Hsb[:, :], H[:, :])
                    nu = min(128, USH - t * 128)
                    nc.sync.dma_start(out=hout_d[t * 128:t * 128 + nu, :],
                                      in_=Hsb[:nu, :])
                    e0 += nch

            def body():
                # ---- social graph first: its decode overlaps the item spmm ----
                if "A" in phases:
                    spmm(ncS, colsS, metaS, WsT_d, Hs_d)

                if "B" in phases:
                    Gs = bpool.tile([128, PB, F], DT)
                    nc.gpsimd.dma_gather(Gs[:, :, :], Hs_d[:, :], binds[:, :],
                                         num_idxs=PMAXg, num_idxs_reg=PMAXg,
                                         elem_size=F)
                    szT = bpool.tile([E, PMAXg], DT)
                    szb = bpool.tile([128, PB, E], DT)
                    zS = bpool.tile([E + 1, PMAXg], BF)   # [s_zT ; ones]
                    nc.vector.memset(zS[E:E + 1, :], 1.0)
                    for bc in range(PB):
                        Ts = psT.tile([128, 128], DT, tag="T")
                        nc.tensor.transpose(Ts[:, :], Gs[:, bc, :], ident[:, :])
                        ts = btmp.tile([E, 128], DT, tag="tq")
                        nc.scalar.activation(ts[:, :], Ts[E:F, :],
                                             mybir.ActivationFunctionType.Exp,
                                             scale=0.5)
                        nc.vector.tensor_add(szT[:, bc * 128:(bc + 1) * 128],
                                             ts[:, :], Ts[0:E, :])
                        tb2 = btmp.tile([128, E], DT, tag="tb")
                        nc.scalar.activation(tb2[:, :], Gs[:, bc, E:F],
                                             mybir.ActivationFunctionType.Exp,
                                             scale=0.5)
                        nc.vector.tensor_add(szb[:, bc, :], tb2[:, :], Gs[:, bc, 0:E])
                    nc.vector.tensor_copy(zS[0:E, :], szT[:, :])
                    for bc in range(PB):
                        nc.sync.dma_start(out=sz_d[bc * 128:(bc + 1) * 128, :],
                                          in_=szb[:, bc, :])
                    if "C" in phases:
                        decode(WspS_d, U, zS, recS_d)

                # ---- item graph ----
                if "A" in phases:
                    spmm(ncA, colsA, metaA, WqT_d, Hq_d)

                if "B" in phases:
                    Gq = bpool.tile([128, PB, F], DT)
                    nc.gpsimd.dma_gather(Gq[:, :, :], Hq_d[:, :], binds[:, :],
                                         num_idxs=PMAXg, num_idxs_reg=PMAXg,
                                         elem_size=F)
                    uzT = bpool.tile([E, PMAXg], DT)
                    uzb = bpool.tile([128, PB, E], DT)
                    for bc in range(PB):
                        Tq = psT.tile([128, 128], DT, tag="T")
                        nc.tensor.transpose(Tq[:, :], Gq[:, bc, :], ident[:, :])
                        tq = btmp.tile([E, 128], DT, tag="tq")
                        nc.scalar.activation(tq[:, :], Tq[E:F, :],
                                             mybir.ActivationFunctionType.Exp,
                                             scale=0.5)
                        nc.vector.tensor_add(uzT[:, bc * 128:(bc + 1) * 128],
                                             tq[:, :], Tq[0:E, :])
                        tb = btmp.tile([128, E], DT, tag="tb")
                        nc.scalar.activation(tb[:, :], Gq[:, bc, E:F],
                                             mybir.ActivationFunctionType.Exp,
                                             scale=0.5)
                        nc.vector.tensor_add(uzb[:, bc, :], tb[:, :], Gq[:, bc, 0:E])
                    for bc in range(PB):
                        nc.sync.dma_start(out=uz_d[bc * 128:(bc + 1) * 128, :],
                                          in_=uzb[:, bc, :])

                    # attention: score = aW2 @ tanh(aW1 @ all_zT + ab1)
                    zcat = bpool.tile([128, PMAXg], DT)
                    nc.vector.tensor_copy(zcat[0:E, :], uzT[:, :])
                    nc.vector.tensor_copy(zcat[E:F, :], szT[:, :])
                    zA = bpool.tile([E + 1, PMAXg], BF)   # [zT ; ones]
                    nc.vector.memset(zA[E:E + 1, :], 1.0)
                    h1 = bpool.tile([E, PMAXg], DT)
                    score = bpool.tile([1, PMAXg], DT)
                    for c0 in range(0, PMAXg, 512):
                        cw = min(512, PMAXg - c0)
                        hp = psM.tile([E, 512], DT, tag="ps")
                        nc.tensor.matmul(hp[:, :cw], attn[:, 0:E],
                                         zcat[:, c0:c0 + cw], start=True, stop=True)
                        nc.scalar.activation(h1[:, c0:c0 + cw], hp[:, :cw],
                                             mybir.ActivationFunctionType.Tanh,
                                             bias=attn[0:E, E:E + 1], scale=1.0)
                    for c0 in range(0, PMAXg, 512):
                        cw = min(512, PMAXg - c0)
                        sp = psM.tile([1, 512], DT, tag="ps")
                        nc.tensor.matmul(sp[0:1, :cw], attn[0:E, E + 1:E + 2],
                                         h1[:, c0:c0 + cw], start=True, stop=True)
                        nc.vector.tensor_copy(score[0:1, c0:c0 + cw], sp[0:1, :cw])
                    ones_row = btmp.tile([1, E], DT, tag="ones")
                    nc.vector.memset(ones_row[:, :], 1.0)
                    d_ = bpool.tile([E, PMAXg], DT)
                    nc.vector.tensor_sub(d_[:, :], uzT[:, :], szT[:, :])
                    for c0 in range(0, PMAXg, 512):
                        cw = min(512, PMAXg - c0)
                        sb = psM.tile([E, 512], DT, tag="ps")
                        nc.tensor.matmul(sb[:, :cw], ones_row[:, :],
                                         score[0:1, c0:c0 + cw], start=True, stop=True)
                        tmp = btmp.tile([E, 512], DT, tag="tmul")
                        nc.vector.tensor_mul(tmp[:, :cw], sb[:, :cw], d_[:, c0:c0 + cw])
                        nc.vector.tensor_add(zA[0:E, c0:c0 + cw], tmp[:, :cw],
                                             szT[:, c0:c0 + cw])
                    if "C" in phases:
                        decode(WpA_d, I, zA, recA_d)
                elif "C" in phases:
                    zA = bpool.tile([E + 1, PMAXg], BF)
                    zS = bpool.tile([E + 1, PMAXg], BF)
                    nc.vector.memset(zA[:, :], 0.5)
                    nc.vector.memset(zS[:, :], 0.5)
                    decode(WspS_d, U, zS, recS_d)
                    decode(WpA_d, I, zA, recA_d)

            def decode(w_d, width, z_sb, out_d):
                for n0 in range(0, width, DEC_N):
                    nw = min(DEC_N, width - n0)
                    wt = wdec.tile([E + 1, DEC_N], BF, tag="w")
                    nc.sync.dma_start(out=wt[:, :nw], in_=w_d[:, n0:n0 + nw])
                    for bc in range(PB):
                        ps = psD.tile([128, DEC_N], DT, tag="d")
                        nc.tensor.matmul(
                            ps[:, :nw],
                            z_sb[:, bc * 128:(bc + 1) * 128],
                            wt[:, :nw], start=True, stop=True)
                        ob = odec.tile([128, DEC_N], DT, tag="o")
                        nc.vector.tensor_copy(ob[:, :nw], ps[:, :nw])
                        nc.sync.dma_start(
                            out=out_d[bc * 128:(bc + 1) * 128, n0:n0 + nw],
                            in_=ob[:, :nw])

            if rep == 1:
                body()
            else:
                with tc.For_i(0, rep, 1):
                    body()

    nc.compile()
    return nc


def _wrap16(idx):
    """int16 index vector -> (128, n/16) wrapped+replicated layout."""
    w = idx.reshape(-1, 16).T.astype(np.int16)
    return np.tile(w, (8, 1))


def _prep_graph(rows, cols, vals, bias_row):
    """Sort edges by (core, user-tile); per tile, pad to a chunk count
    uniform across cores; emit per-core cols/lrow/val tables."""
    core = rows // USH
    lrow_g = rows % USH
    tl = lrow_g // 128
    lr = (lrow_g % 128).astype(np.float32)
    order = np.lexsort((tl, core))
    core, tl, lr = core[order], tl[order], lr[order]
    cols, vals = cols[order], vals[order]

    # counts per (core, tile), including bias edges
    cnt = np.zeros((NC, UT), np.int64)
    np.add.at(cnt, (core, tl), 1)
    nu = np.minimum(128, USH - np.arange(UT) * 128)
    cntb = cnt + nu[None, :]
    nch = ((cntb.max(axis=0) + 127) // 128).astype(np.int64)   # chunks per tile
    tot = int(nch.sum())

    colsT = np.zeros((NC, tot * 128), np.int16)
    lrowT = np.zeros((NC, tot * 128), np.float32)
    valT = np.zeros((NC, tot * 128), np.float32)
    # edge segment starts per (core, tile) in the sorted arrays
    seg = np.zeros((NC, UT + 1), np.int64)
    for c in range(NC):
        m = core == c
        seg[c, 1:] = np.cumsum(np.bincount(tl[m], minlength=UT))
    offs = np.concatenate([[0], np.cumsum(nch * 128)])
    for c in range(NC):
        m = core == c
        ctl, ccols, cvals, clr = tl[m], cols[m], vals[m], lr[m]
        for t in range(UT):
            s, e = seg[c, t], seg[c, t + 1]
            k = e - s
            o = offs[t]
            colsT[c, o:o + k] = ccols[s:e]
            valT[c, o:o + k] = cvals[s:e]
            lrowT[c, o:o + k] = clr[s:e]
            # bias edges
            nb = int(nu[t])
            colsT[c, o + k:o + k + nb] = bias_row
            valT[c, o + k:o + k + nb] = 1.0
            lrowT[c, o + k:o + k + nb] = np.arange(nb, dtype=np.float32)
    return nch, colsT, lrowT, valT


def _pack_meta(nch, lrowT, valT):
    tot = int(nch.sum())
    metas = []
    iota = np.tile(np.arange(128, dtype=np.float32)[None, :], (128, 1))
    for c in range(NC):
        m = np.empty((128, 128 + 2 * tot), np.float32)
        m[:, 0:128] = iota
        m[:, 128:128 + tot] = lrowT[c].reshape(tot, 128).T
        m[:, 128 + tot:] = valT[c].reshape(tot, 128).T
        metas.append(m)
    return metas


def kernel(inputs, graph_row, graph_col, graph_val, s_row, s_col, s_val,
           Wq, bq, Ws, bs, aW1, ab1, aW2, Wp, bp, Wsp, bsp):
    inputs = np.asarray(inputs)
    graph_row = np.asarray(graph_row).astype(np.int64)
    graph_col = np.asarray(graph_col).astype(np.int64)
    graph_val = np.asarray(graph_val, dtype=np.float32)
    s_row = np.asarray(s_row).astype(np.int64)
    s_col = np.asarray(s_col).astype(np.int64)
    s_val = np.asarray(s_val, dtype=np.float32)
    Wq = np.asarray(Wq, dtype=np.float32)
    bq = np.asarray(bq, dtype=np.float32)
    Ws = np.asarray(Ws, dtype=np.float32)
    bs = np.asarray(bs, dtype=np.float32)
    aW1 = np.asarray(aW1, dtype=np.float32)
    ab1 = np.asarray(ab1, dtype=np.float32)
    aW2 = np.asarray(aW2, dtype=np.float32)
    Wp = np.asarray(Wp, dtype=np.float32)
    bp = np.asarray(bp, dtype=np.float32)
    Wsp = np.asarray(Wsp, dtype=np.float32)
    bsp = np.asarray(bsp, dtype=np.float32)

    ncA, colsA, lrowA, valA = _prep_graph(graph_row, graph_col, graph_val, I)
    ncS, colsS, lrowS, valS = _prep_graph(s_row, s_col, s_val, U)
    metasA = _pack_meta(ncA, lrowA, valA)
    metasS = _pack_meta(ncS, lrowS, valS)

    # batch grouping by owner core
    bi = inputs.astype(np.int64)
    owner = bi // USH
    pos_by_core = [np.nonzero(owner == c)[0] for c in range(NC)]
    nmax = max(len(p) for p in pos_by_core)
    PB = (nmax + 127) // 128
    PMAXg = PB * 128

    binds_maps = []
    for c in range(NC):
        lu = np.zeros(PMAXg, np.int16)
        p = pos_by_core[c]
        lu[:len(p)] = (bi[p] - c * USH).astype(np.int16)
        binds_maps.append(_wrap16(lu))

    import ml_dtypes
    WqT_aug = np.vstack([Wq.T, bq[None, :]]).astype(ml_dtypes.bfloat16)
    WsT_aug = np.vstack([Ws.T, bs[None, :]]).astype(ml_dtypes.bfloat16)
    WpA = np.vstack([Wp.T, bp[None, :]]).astype(ml_dtypes.bfloat16)
    WspS = np.vstack([Wsp.T, bsp[None, :]]).astype(ml_dtypes.bfloat16)
    ident = np.eye(128, dtype=np.float32)
    attn = np.zeros((128, E + 2), np.float32)
    attn[:, 0:E] = aW1.T            # (2E x E)
    attn[0:E, E] = ab1
    attn[0:E, E + 1] = aW2[0]

    key = (tuple(int(x) for x in ncA), tuple(int(x) for x in ncS), PB)
    if key not in _cache:
        _cache[key] = _build(list(map(int, ncA)), list(map(int, ncS)), PB)
    nc = _cache[key]

    in_maps = []
    for c in range(NC):
        in_maps.append(dict(
            WqT=WqT_aug, WsT=WsT_aug,
            colsA=_wrap16(colsA[c]), colsS=_wrap16(colsS[c]),
            metaA=metasA[c], metaS=metasS[c],
            binds=binds_maps[c], ident=ident, attn=attn,
            WpA=WpA, WspS=WspS,
        ))

    global _last_in_maps
    _last_in_maps = in_maps
    res = run_bass_kernel_spmd(nc, in_maps, core_ids=list(range(NC)))
    results = res.results

    mu = np.concatenate([results[c]["Hq"][:, :E] for c in range(NC)], 0)
    logvar = np.concatenate([results[c]["Hq"][:, E:] for c in range(NC)], 0)
    s_mu = np.concatenate([results[c]["Hs"][:, :E] for c in range(NC)], 0)
    s_logvar = np.concatenate([results[c]["Hs"][:, E:] for c in range(NC)], 0)
    u_z = np.empty((B, E), np.float32)
    s_z = np.empty((B, E), np.float32)
    recon_A = np.empty((B, I), np.float32)
    recon_S = np.empty((B, U), np.float32)
    for c in range(NC):
        p = pos_by_core[c]
        n = len(p)
        u_z[p] = results[c]["uz"][:n]
        s_z[p] = results[c]["sz"][:n]
        recon_A[p] = results[c]["recA"][:n]
        recon_S[p] = results[c]["recS"][:n]
    return (recon_A, recon_S, mu, logvar, s_mu, s_logvar, u_z, s_z)
